# revision 5
# baseline (speedup 1.0000x reference)
"""Trainium2 Bass kernel for teacher-forced GRU decoder (nn_DecoderRNN).

Problem (hardcoded from spec):
  B=2048, T=160, H=512, EMB=64, V=128, SOS=0
  reference computes, per step t (tokens teacher-forced, x = relu(emb[tok])):
      gi = x @ W_ih.T + b_ih ; gh = h @ W_hh.T + b_hh
      r = sig(gi_r + gh_r); z = sig(gi_z + gh_z); n = tanh(gi_n + r*gh_n)
      h = (1-z)*n + z*h ; logits = h @ W_out.T + b_out
  outputs: log_softmax(logits) [T, B, V] and final hidden [1, B, H].

Strategy (8 cores, data-parallel over batch, 256 rows/core):
  - Everything transposed on-chip: hidden state hT stored [128 part, 4, 256]
    (partition = H row within chunk, free = batch), so the recurrent matmul
    needs no per-step transposes and gates are elementwise in this layout.
  - Since relu(embedding) has only V=128 distinct rows, precompute (on device,
    fp32) a table giT[v, 3H] = relu(emb)[v] @ W_ih.T + b_ih (+ b_hh folded in
    for the r/z parts).  Per step, gi rows are "gathered" via a one-hot
    matmul that accumulates directly into PSUM with the W_hh matmuls.
  - Per-step matmuls in float32r (full PE speed, ~1e-4 rel err).
  - Raw logits stored to HBM per step; log_softmax applied in a final phase
    (avoids thrashing ACT tables between sigmoid/tanh and exp/ln sets).
"""
import sys

sys.path.insert(0, "/opt/trn_rl_repo")

import numpy as np

import concourse.bass as bass
import concourse.mybir as mybir
import concourse.tile as tile
from concourse import bacc
from concourse.bass_utils import run_bass_kernel_spmd

F32 = mybir.dt.float32
AF = mybir.ActivationFunctionType
OP = mybir.AluOpType

B = 2048
T = 160
H = 512
EMB = 64
V = 128
NCORES = 8
BL = B // NCORES          # 256 batch rows per core
KC = H // 128             # 4 hidden chunks
NC3 = 3 * H // 128        # 12 output chunks of 3H

LAST_EXEC_NS = None
_CACHE = {}


def build(T_steps=T, mm_dtype=mybir.dt.float32r, trace=False):
    DT = mm_dtype
    nc = bacc.Bacc("TRN2", target_bir_lowering=False, debug=False)

    # ---------------- I/O ----------------
    h0t = nc.dram_tensor("h0t", [128, KC, BL], DT, kind="ExternalInput")
    onehot = nc.dram_tensor("onehot", [T_steps, 128, BL], DT, kind="ExternalInput")
    whht = nc.dram_tensor("whht", [128, KC, NC3, 128], DT, kind="ExternalInput")
    embt = nc.dram_tensor("embt", [EMB, V], F32, kind="ExternalInput")
    wih_aug = nc.dram_tensor("wih_aug", [128, 3 * H], F32, kind="ExternalInput")
    bhhn = nc.dram_tensor("bhhn", [1, H], DT, kind="ExternalInput")
    woutt = nc.dram_tensor("woutt", [128, KC, V], DT, kind="ExternalInput")
    bout = nc.dram_tensor("bout", [1, V], DT, kind="ExternalInput")

    logp = nc.dram_tensor("logp", [T_steps, BL, V], F32, kind="ExternalOutput")
    hlast = nc.dram_tensor("hlast", [128, KC, BL], DT, kind="ExternalOutput")

    with tile.TileContext(nc) as tc:
        with (
            tc.tile_pool(name="consts", bufs=1) as consts,
            tc.tile_pool(name="hpool", bufs=3) as hpool,
            tc.tile_pool(name="gates", bufs=2) as gates,
            tc.tile_pool(name="ohp", bufs=4) as ohp,
            tc.tile_pool(name="lsb", bufs=3) as lsbp,
            tc.tile_pool(name="psb", bufs=2, space="PSUM") as psb,
            tc.tile_pool(name="dram", bufs=1, space="DRAM") as dramp,
            tc.tile_pool(name="fin", bufs=4) as fin,
        ):
            # ------------- load constants -------------
            whht_sb = consts.tile([128, KC, NC3, 128], DT)
            nc.sync.dma_start(whht_sb[:], whht[:])
            woutt_sb = consts.tile([128, KC, V], DT)
            nc.sync.dma_start(woutt_sb[:], woutt[:])
            bout_sb = consts.tile([1, V], DT)
            nc.sync.dma_start(bout_sb[:], bout[:])
            bhhn_sb = consts.tile([1, H], DT)
            nc.sync.dma_start(bhhn_sb[:], bhhn[:])
            wih_sb = consts.tile([128, 3 * H], F32)
            nc.sync.dma_start(wih_sb[:], wih_aug[:])
            embt_sb = consts.tile([EMB, V], F32)
            nc.sync.dma_start(embt_sb[:], embt[:])

            ones_f = consts.tile([1, BL], F32)
            nc.vector.memset(ones_f[:], 1.0)
            ones_sb = consts.tile([1, BL], DT)
            nc.vector.tensor_copy(ones_sb[:], ones_f[:])
            ones128_sb = consts.tile([1, 128], DT)
            nc.vector.tensor_copy(ones128_sb[:], ones_f[:, :128])

            # ------------- giT table (one-time, fp32 exact) -------------
            # lhsT_aug[k, v] = relu(emb)[v, k] for k<EMB ; 1.0 at k=EMB ; 0 else
            lhsT_aug = consts.tile([128, V], F32)
            nc.vector.memset(lhsT_aug[:], 0.0)
            nc.vector.tensor_scalar_max(lhsT_aug[:EMB, :], embt_sb[:], 0.0)
            nc.vector.memset(lhsT_aug[EMB : EMB + 1, :], 1.0)

            giT = consts.tile([128, NC3, 128], DT)  # [v, n-chunk, n-in-chunk]
            for nn in range(3):
                pg = psb.tile([128, 8, BL], F32, tag="big")
                nc.tensor.matmul(
                    pg[:, 0:2, :].rearrange("p a b -> p (a b)"),
                    lhsT_aug[:],
                    wih_sb[:, nn * 512 : (nn + 1) * 512],
                    start=True,
                    stop=True,
                )
                nc.scalar.copy(
                    giT[:, nn * 4 : (nn + 1) * 4, :].rearrange("p a b -> p (a b)"),
                    pg[:, 0:2, :].rearrange("p a b -> p (a b)"),
                )

            def giT_l(n_idx):
                return giT[:, n_idx, :]

            # ------------- initial hidden -------------
            hT = hpool.tile([128, KC, BL], DT, tag="h")
            nc.sync.dma_start(hT[:], h0t[:])

            logits_dram = dramp.tile([T_steps, BL, V], F32)

            # ------------- recurrence -------------
            # Per-step gate math at half granularity (H chunks 01 / 23) so the
            # serial chain sig(r) -> u -> v -> tanh -> m1n -> h_new pipelines
            # against the PE matmul stream and the other half.
            for t in range(T_steps):
                oh = ohp.tile([128, BL], DT, tag="oh")
                nc.sync.dma_start(oh[:], onehot[t])

                hr = hT[:]

                ps_rz = psb.tile([128, 8, BL], F32, tag="big")
                ps_b = psb.tile([128, 8, BL], F32, tag="big")

                def mm_group(ps, g, n_idx):
                    nc.tensor.matmul(
                        ps[:, g, :], giT_l(n_idx), oh[:], start=True, stop=False
                    )
                    for k in range(KC):
                        nc.tensor.matmul(
                            ps[:, g, :],
                            whht_sb[:, k, n_idx, :],
                            hr[:, k, :],
                            start=False,
                            stop=(k == KC - 1),
                        )

                def mm_hn(c):
                    for k in range(KC):
                        nc.tensor.matmul(
                            ps_b[:, 4 + c, :],
                            whht_sb[:, k, 8 + c, :],
                            hr[:, k, :],
                            start=(k == 0),
                            stop=False,
                        )
                    # + b_hh_n broadcast over batch (rank-1 via K=1 matmul)
                    nc.tensor.matmul(
                        ps_b[:, 4 + c, :],
                        bhhn_sb[0:1, c * 128 : (c + 1) * 128],
                        ones_sb[:],
                        start=False,
                        stop=True,
                    )

                r_sb = gates.tile([128, KC, BL], F32, tag="r")
                z_sb = gates.tile([128, KC, BL], F32, tag="z")
                n_sb = gates.tile([128, KC, BL], F32, tag="n")
                hT_new = hpool.tile([128, KC, BL], DT, tag="h")

                for hf in range(2):  # half = chunks [2*hf, 2*hf+1]
                    c0, c1 = 2 * hf, 2 * hf + 2
                    # A: r chunks then z chunks of this half
                    for c in range(c0, c1):
                        mm_group(ps_rz, c, c)          # a_r chunk c
                    # B: h_n chunks of this half (feeds u early)
                    for c in range(c0, c1):
                        mm_hn(c)
                    for c in range(c0, c1):
                        nc.tensor.matmul(               # i_n chunk c
                            ps_b[:, c, :], giT_l(8 + c), oh[:], start=True, stop=True
                        )
                    for c in range(c0, c1):
                        mm_group(ps_rz, 4 + c, 4 + c)  # a_z chunk c

                    rh = r_sb[:, c0:c1, :]
                    nc.scalar.activation(rh, ps_rz[:, c0:c1, :], AF.Sigmoid)
                    zh = z_sb[:, c0:c1, :]
                    nc.scalar.activation(zh, ps_rz[:, 4 + c0 : 4 + c1, :], AF.Sigmoid)

                    u_sb = gates.tile([128, 2, BL], F32, tag="u")
                    nc.vector.tensor_tensor(
                        u_sb[:], ps_b[:, 4 + c0 : 4 + c1, :], rh, OP.mult
                    )
                    v_sb = gates.tile([128, 2, BL], F32, tag="v")
                    nc.vector.tensor_tensor(v_sb[:], u_sb[:], ps_b[:, c0:c1, :], OP.add)
                    nh = n_sb[:, c0:c1, :]
                    nc.scalar.activation(nh, v_sb[:], AF.Tanh)

                    # m2 = z*h (off-chain), m1n = (z-1)*n, h_new = m2 - m1n
                    m2_sb = gates.tile([128, 2, BL], F32, tag="m2")
                    nc.vector.tensor_tensor(
                        m2_sb[:], zh, hT[:, c0:c1, :].bitcast(F32), OP.mult
                    )
                    m1n_sb = gates.tile([128, 2, BL], F32, tag="m1n")
                    nc.vector.scalar_tensor_tensor(
                        m1n_sb[:], zh, 1.0, nh, OP.subtract, OP.mult
                    )
                    nc.vector.tensor_tensor(
                        hT_new[:, c0:c1, :], m2_sb[:], m1n_sb[:], OP.subtract
                    )

                # logits [b, v] : lhsT = hT_new chunks (stationary), rhs = WoutT
                hnr = hT_new[:]
                ps_l = psb.tile([128, 2, V], F32, tag="big")
                for bc in range(2):
                    for k in range(KC):
                        nc.tensor.matmul(
                            ps_l[:, bc, :],
                            hnr[:, k, bc * 128 : (bc + 1) * 128],
                            woutt_sb[:, k, :],
                            start=(k == 0),
                            stop=False,
                        )
                    nc.tensor.matmul(
                        ps_l[:, bc, :],
                        ones128_sb[:],
                        bout_sb[:],
                        start=False,
                        stop=True,
                    )

                logit_sb = lsbp.tile([128, 2, V], F32, tag="lg")
                nc.scalar.copy(logit_sb[:], ps_l[:])
                nc.sync.dma_start(
                    logits_dram[t].rearrange("(c p) v -> p c v", p=128), logit_sb[:]
                )

                hT = hT_new

            nc.sync.dma_start(hlast[:], hT[:])

            # ------------- final phase: log_softmax -------------
            rows = T_steps * BL
            lp_flat = logp.ap().rearrange("t b v -> (t b) v")
            ld_flat = logits_dram[:].rearrange("t b v -> (t b) v")
            G = 4  # tiles of [128, G, V]
            ngroups = rows // (128 * G)
            for i in range(ngroups):
                l_sb = fin.tile([128, G, V], F32, tag="fl")
                nc.sync.dma_start(
                    l_sb[:],
                    ld_flat[i * 128 * G : (i + 1) * 128 * G, :].rearrange(
                        "(c p) v -> p c v", p=128
                    ),
                )
                e_sb = fin.tile([128, G, V], F32, tag="fe")
                nc.scalar.activation(e_sb[:], l_sb[:], AF.Exp)
                s_sb = fin.tile([128, G], F32, tag="fs")
                nc.vector.tensor_reduce(s_sb[:], e_sb[:], mybir.AxisListType.X, OP.add)
                ls_sb = fin.tile([128, G], F32, tag="fls")
                nc.scalar.activation(ls_sb[:], s_sb[:], AF.Ln)
                o_sb = fin.tile([128, G, V], F32, tag="fo")
                for c in range(G):
                    nc.vector.tensor_scalar(
                        o_sb[:, c, :],
                        l_sb[:, c, :],
                        ls_sb[:, c : c + 1],
                        None,
                        OP.subtract,
                    )
                nc.sync.dma_start(
                    lp_flat[i * 128 * G : (i + 1) * 128 * G, :].rearrange(
                        "(c p) v -> p c v", p=128
                    ),
                    o_sb[:],
                )

    nc.compile()
    return nc


def _prep_host(encoder_hidden, target_tensor, embedding, W_ih, W_hh, b_ih, b_hh,
               W_out, b_out, T_steps=T):
    """Build per-core input maps (host-side data marshalling only)."""
    f32 = np.float32
    # teacher-forced tokens [T, B]
    tok = np.concatenate(
        [np.zeros((B, 1), dtype=target_tensor.dtype), target_tensor[:, : T - 1]],
        axis=1,
    ).T[:T_steps]  # [T_steps, B]

    eye = np.eye(V, dtype=f32)
    whht_np = np.ascontiguousarray(
        W_hh.T.astype(f32).reshape(KC, 128, NC3, 128).transpose(1, 0, 2, 3)
    )
    wih_aug_np = np.zeros((128, 3 * H), dtype=f32)
    wih_aug_np[:EMB] = W_ih.T.astype(f32)
    btot = b_ih.astype(f32).copy()
    btot[: 2 * H] += b_hh[: 2 * H].astype(f32)
    wih_aug_np[EMB] = btot
    embt_np = np.ascontiguousarray(embedding.T.astype(f32))
    bhhn_np = np.ascontiguousarray(b_hh[2 * H :].astype(f32).reshape(1, H))
    woutt_np = np.ascontiguousarray(
        W_out.T.astype(f32).reshape(KC, 128, V).transpose(1, 0, 2)
    )
    bout_np = np.ascontiguousarray(b_out.astype(f32).reshape(1, V))

    in_maps = []
    for c in range(NCORES):
        b0 = c * BL
        h0 = encoder_hidden[0, b0 : b0 + BL, :].astype(f32)  # [BL, H]
        h0t_np = np.ascontiguousarray(
            h0.T.reshape(KC, 128, BL).transpose(1, 0, 2)
        )  # [128, KC, BL]
        oh_np = np.ascontiguousarray(
            eye[tok[:, b0 : b0 + BL]].transpose(0, 2, 1)
        )  # [T, V=128, BL]
        in_maps.append(
            {
                "h0t": h0t_np,
                "onehot": oh_np,
                "whht": whht_np,
                "embt": embt_np,
                "wih_aug": wih_aug_np,
                "bhhn": bhhn_np,
                "woutt": woutt_np,
                "bout": bout_np,
            }
        )
    return in_maps


def kernel(encoder_outputs, encoder_hidden, target_tensor, embedding, W_ih, W_hh,
           b_ih, b_hh, W_out, b_out, _trace=False, _T=T):
    global LAST_EXEC_NS
    encoder_hidden = np.asarray(encoder_hidden)
    target_tensor = np.asarray(target_tensor)
    key = (_T, _trace)
    if key not in _CACHE:
        _CACHE[key] = build(T_steps=_T, trace=_trace)
    nc = _CACHE[key]

    in_maps = _prep_host(
        encoder_hidden, target_tensor, np.asarray(embedding), np.asarray(W_ih),
        np.asarray(W_hh), np.asarray(b_ih), np.asarray(b_hh), np.asarray(W_out),
        np.asarray(b_out), T_steps=_T,
    )
    res = run_bass_kernel_spmd(
        nc, in_maps, core_ids=list(range(NCORES)), trace=_trace
    )
    LAST_EXEC_NS = res.exec_time_ns

    log_probs = np.empty((_T, B, V), dtype=np.float32)
    hidden = np.empty((1, B, H), dtype=np.float32)
    for c in range(NCORES):
        b0 = c * BL
        out = res.results[c]
        log_probs[:, b0 : b0 + BL, :] = out["logp"]
        hl = out["hlast"]  # [128, KC, BL]
        hidden[0, b0 : b0 + BL, :] = hl.transpose(1, 0, 2).reshape(H, BL).T
    return log_probs, hidden
